# revision 18
# baseline (speedup 1.0000x reference)
"""Multi-head causal attention (B=8, T=2048, C=384, H=6, Dh=64) on 8 TRN2 cores.

Sharding: data-parallel over batch - core b computes batch element b end to end
(no collectives).

v2 pipeline design (vs v1 baseline at ~300us):
- S-score psum tiles hold PAIRS of s-chunks [128, 2, 512] spanning 2 psum
  banks; ONE exp (ACT) instruction covers both chunks -> halves ACT
  per-instruction overhead (240 -> 120 exp instrs).
- exp APs are fringe-trimmed ([.., d0:512]) and PV matmuls stream only the
  causal columns (N = 512-d), removing the P memsets entirely.
- software-pipelined issue order per head: S(p) / exp(p) / PV(p-1) so the
  tensor engine always has queued work while ACT computes exp.
- denominator row (augmented-V row 64 of O) is copied by GPSIMD into a
  per-block [6, 512] tile; 4 batched transposes [6,128]->[128,6] per q-block
  replace 24 single-column transposes.
- output projection per (j,q): 6 per-head K=64 matmuls; normalization via
  scalar_tensor_tensor with per-partition reciprocal denominators, split
  across DVE (h=0,2,4,5 + final add) and GPSIMD (h=1,3) dual accumulators.
- phases interleaved across q-blocks: QKV projections for block j+1 are
  issued inside attention of block j; output projection of block j-1 is
  issued between attention heads of block j.

Per-core layout (all "T" means transposed, head-dim/channel on partitions):
  xT   [128, 3, 2048]  bf16   c = 128*ci + p
  wq/wk[128, 3, 384]   bf16   packed Wq[h,c,d] -> [c, h*64+d]
  wv   [128, 3, 384]   bf16
  wp   [128, 3, 384]   bf16   Wp[c, e] -> [128, ci, e]
  biasb[128, 384]      f32    bias broadcast rows
  QT/KT/attT [128, 3, 2048] bf16  (hd = 128*bi + po + d, po = (h%2)*64)
  Vt   [128, 16, 6, 65] bf16  V augmented with ones col (softmax denom row)
"""

import numpy as np
import ml_dtypes

import concourse.bass as bass
import concourse.tile as tile
from concourse import bacc, mybir
from concourse.bass import ts, ds

F32 = mybir.dt.float32
BF16 = mybir.dt.bfloat16
FP8 = mybir.dt.float8e4
DBLROW = mybir.MatmulPerfMode.DoubleRow
AF = mybir.ActivationFunctionType
ALU = mybir.AluOpType

B, T, C = 8, 2048, 384
H, DH = 6, 64
SCALE = DH ** -0.5
NCORES = 8
TJ = 512            # q-block width
NJ = T // TJ        # 4 q-blocks
SC = 128            # s-chunk
NQ = TJ // SC       # q-sub-chunks / s-chunks per block (4)
NCI = C // 128      # 3 channel chunks


def build_kernel():
    nc = bacc.Bacc("TRN2", target_bir_lowering=False, debug=False)

    xT_d = nc.dram_tensor("xT", [128, NCI, T], BF16, kind="ExternalInput").ap()
    wq_d = nc.dram_tensor("wq", [128, NCI, C], BF16, kind="ExternalInput").ap()
    wk_d = nc.dram_tensor("wk", [128, NCI, C], BF16, kind="ExternalInput").ap()
    wv_d = nc.dram_tensor("wv", [128, NCI, C], BF16, kind="ExternalInput").ap()
    wp_d = nc.dram_tensor("wp", [128, NCI, C], BF16, kind="ExternalInput").ap()
    biasb_d = nc.dram_tensor("biasb", [128, C], F32, kind="ExternalInput").ap()
    iden_d = nc.dram_tensor("iden", [128, 128], F32, kind="ExternalInput").ap()
    y_d = nc.dram_tensor("y", [T, C], F32, kind="ExternalOutput").ap()

    with tile.TileContext(nc) as tc:
        with tc.tile_pool(name="const", bufs=1) as cpool, \
             tc.tile_pool(name="ps", bufs=1, space="PSUM") as ps, \
             tc.tile_pool(name="pp", bufs=3) as ppool, \
             tc.tile_pool(name="yp", bufs=2) as ypool:
            xT = cpool.tile([128, NCI, T], BF16)
            wq = cpool.tile([128, NCI, C], BF16)
            wk = cpool.tile([128, NCI, C], BF16)
            wv = cpool.tile([128, NCI, C], BF16)
            wp = cpool.tile([128, NCI, C], BF16)
            biasb = cpool.tile([128, C], F32)
            iden = cpool.tile([128, 128], F32)
            QT = cpool.tile([128, NCI, T], BF16)
            KT = cpool.tile([128, NCI, T], BF16)
            attT = cpool.tile([128, NCI, T], BF16)
            # V augmented with ones col: bf16 copy for block j=0 (first 4
            # s-chunks), fp8 e4m3 copy for the DoubleRow PV of blocks j>=1
            Vt = cpool.tile([128, NQ, H, 65], BF16)
            # head-major, 80B chunk pitch: DoubleRow k-tile stride must be
            # 16B-aligned (s3_lw_dual_fp8_restrictions)
            Vt8 = cpool.tile([128, H, 16, 80], FP8)
            onesb = cpool.tile([128, 16 * H], BF16)
            onesb8 = cpool.tile([128, 16 * H], FP8)

            for ci in range(NCI):
                nc.sync.dma_start(xT[:, ci, :], xT_d[:, ci, :])
            nc.sync.dma_start(wq[:], wq_d[:])
            nc.sync.dma_start(wk[:], wk_d[:])
            nc.sync.dma_start(wv[:], wv_d[:])
            nc.sync.dma_start(wp[:], wp_d[:])
            nc.sync.dma_start(biasb[:], biasb_d[:])
            nc.sync.dma_start(iden[:], iden_d[:])
            # augmented-ones column of Vt/Vt8 (col 64 of each head slot)
            nc.gpsimd.memset(onesb[:], 1.0)
            nc.gpsimd.memset(onesb8[:], 1.0)
            nc.vector.tensor_copy(
                Vt[:, :, :, 64:65],
                onesb[:, 0:NQ * H].rearrange("p (a b c) -> p a b c", a=NQ, b=H),
            )
            nc.vector.tensor_copy(
                Vt8[:, :, :, 64:65],
                onesb8[:].rearrange("p (a b c) -> p a b c", a=H, b=16),
            )

            def proj_block(jb):
                """QT/KT for t-block jb; V rows for s-chunks 4jb..4jb+3."""
                for dst, w in ((QT, wq), (KT, wk)):
                    for pi in range(NCI):
                        pt = ps.tile([128, TJ], F32, tag="mm", bufs=3,
                                     name=f"pqk{jb}{pi}")
                        for ci in range(NCI):
                            nc.tensor.matmul(
                                pt[:],
                                lhsT=w[:, ci, ts(pi, 128)],
                                rhs=xT[:, ci, ts(jb, TJ)],
                                start=(ci == 0), stop=(ci == NCI - 1),
                            )
                        nc.vector.tensor_copy(dst[:, pi, ts(jb, TJ)], pt[:])
                for si in range(NQ * jb, NQ * jb + NQ):
                    pt = ps.tile([128, C], F32, tag="mm", bufs=3,
                                 name=f"pv{si}")
                    for ci in range(NCI):
                        nc.tensor.matmul(
                            pt[:],
                            lhsT=xT[:, ci, ts(si, 128)],
                            rhs=wv[:, ci, :],
                            start=(ci == 0), stop=(ci == NCI - 1),
                        )
                    nc.vector.tensor_copy(
                        Vt8[:, :, si, 0:64],
                        pt[:].rearrange("p (h d) -> p h d", h=H),
                    )
                    if jb == 0:
                        nc.vector.tensor_copy(
                            Vt[:, si, :, 0:64],
                            pt[:].rearrange("p (h d) -> p h d", h=H),
                        )

            def attention_head(j, h, dstage):
                po = (h % 2) * 64
                bi = h // 2
                nch = NQ * j + NQ       # s-chunks (always even)
                npair = nch // 2
                O = ps.tile([65, TJ], F32, tag="O", bufs=1, name=f"O{j}{h}")
                sps_t = [None] * npair
                P_t = [None] * npair

                def off(i):
                    return SC * i - TJ * j if i >= NQ * j else 0

                def S_pair(p):
                    spt = ps.tile([128, 2, TJ], F32, tag="sp", bufs=2,
                                  name=f"sp{j}{h}{p}")
                    sps_t[p] = spt
                    for c in (0, 1):
                        i = 2 * p + c
                        d = off(i)
                        nc.tensor.matmul(
                            spt[:, c, d:TJ],
                            lhsT=KT[po:po + 64, bi, ts(i, SC)],
                            rhs=QT[po:po + 64, bi, ds(j * TJ + d, TJ - d)],
                            start=True, stop=True,
                        )

                fp8 = j >= 1  # DoubleRow PV for blocks t >= 512

                def EXP_pair(p):
                    d0 = off(2 * p)
                    if fp8:
                        pt = ppool.tile([128, 2, TJ], FP8, tag="P8",
                                        name=f"P{j}{h}{p}")
                    else:
                        pt = ppool.tile([128, 2, TJ], BF16, tag="P",
                                        name=f"P{j}{h}{p}")
                    P_t[p] = pt
                    nc.scalar.activation(pt[:, :, d0:TJ], sps_t[p][:, :, d0:TJ],
                                         AF.Exp, scale=SCALE)
                    if fp8 and 2 * p >= NQ * j:
                        # chunk 1's columns [d0:d0+128) hold garbage exp
                        # values that DoubleRow would fold into the output
                        nc.gpsimd.memset(pt[:, 1, d0:d0 + 128], 0.0)
                    for c in (0, 1):
                        i = 2 * p + c
                        if i >= NQ * j:  # diagonal window mask
                            d = off(i)
                            nc.gpsimd.affine_select(
                                out=pt[:, c, d:d + 128], in_=pt[:, c, d:d + 128],
                                pattern=[[1, 128]],
                                compare_op=ALU.is_ge,
                                fill=0.0, base=0, channel_multiplier=-1,
                            )

                def PV_pair(p):
                    if fp8:
                        d0 = off(2 * p)
                        nc.tensor.matmul(
                            O[:, d0:TJ],
                            lhsT=Vt8[:, h, 2 * p:2 * p + 2, 0:65],
                            rhs=P_t[p][:, :, d0:TJ],
                            start=(p == 0), stop=(p == npair - 1),
                            perf_mode=DBLROW,
                        )
                    else:
                        for c in (0, 1):
                            i = 2 * p + c
                            d = off(i)
                            nc.tensor.matmul(
                                O[:, d:TJ],
                                lhsT=Vt[:, i, h, :],
                                rhs=P_t[p][:, c, d:TJ],
                                start=(i == 0), stop=(i == nch - 1),
                            )

                S_pair(0)
                EXP_pair(0)
                for p in range(1, npair):
                    S_pair(p)
                    EXP_pair(p)
                    PV_pair(p - 1)
                PV_pair(npair - 1)
                # stage unnormalized attT (bf16) and the denominator row
                nc.vector.tensor_copy(attT[po:po + 64, bi, ts(j, TJ)], O[0:64, :])
                nc.vector.tensor_copy(dstage[0:1, h, :], O[64:65, :])

            def transp_recip(j, dstage):
                # scatter the 6 staged denominator rows onto partitions 0..5
                denoms = ypool.tile([H, TJ], F32, tag="denoms", name=f"den{j}")
                nc.sync.dma_start(denoms[0:H, :], dstage[0:1, :, :])
                dT = ps.tile([128, NQ, H], F32, tag="mm", bufs=3, name=f"dT{j}")
                for qq in range(NQ):
                    nc.tensor.transpose(dT[:, qq, :], denoms[0:H, ts(qq, 128)],
                                        iden[0:H, 0:H])
                rT = ypool.tile([128, NQ, H], F32, tag="rT", name=f"rT{j}")
                nc.vector.reciprocal(rT[:], dT[:])
                return rT

            def out_proj_q(j, q, rT):
                tb = NQ * j + q
                Ye = ypool.tile([128, C], F32, tag="Ye", name=f"Ye{tb}")
                for h in range(H):
                    po = (h % 2) * 64
                    bi = h // 2
                    U = ps.tile([128, C], F32, tag="mm", bufs=3,
                                name=f"U{tb}{h}")
                    nc.tensor.matmul(
                        U[:],
                        lhsT=attT[po:po + 64, bi, ts(tb, 128)],
                        rhs=wp[po:po + 64, bi, :],
                        start=True, stop=True,
                    )
                    sc = rT[:, q, h:h + 1]
                    nc.vector.scalar_tensor_tensor(
                        out=Ye[:], in0=U[:], scalar=sc,
                        in1=(biasb[:] if h == 0 else Ye[:]),
                        op0=ALU.mult, op1=ALU.add)
                nc.sync.dma_start(y_d[ts(tb, 128), :], Ye[:])

            def out_proj_head(j, h, dstage, Ys):
                """Per-head output projection for the LAST block: issue as
                soon as head h's attention is done so no work is left for a
                serial tail. Tiny per-head transposes replace the batched
                ones; Ys are the 4 per-q accumulators (DMA'd after h==5)."""
                dTh = ps.tile([128, NQ], F32, tag="mm", bufs=3,
                              name=f"dTh{j}{h}")
                for qq in range(NQ):
                    nc.tensor.transpose(dTh[:, qq:qq + 1],
                                        dstage[0:1, h, ts(qq, 128)],
                                        iden[0:1, 0:1])
                rh = ypool.tile([128, NQ], F32, tag="rh", name=f"rh{j}{h}")
                nc.vector.reciprocal(rh[:], dTh[:])
                po = (h % 2) * 64
                bi = h // 2
                for q in range(NQ):
                    tb = NQ * j + q
                    U = ps.tile([128, C], F32, tag="mm", bufs=3,
                                name=f"Ul{tb}{h}")
                    nc.tensor.matmul(
                        U[:],
                        lhsT=attT[po:po + 64, bi, ts(tb, 128)],
                        rhs=wp[po:po + 64, bi, :],
                        start=True, stop=True,
                    )
                    nc.vector.scalar_tensor_tensor(
                        out=Ys[q][:], in0=U[:], scalar=rh[:, q:q + 1],
                        in1=(biasb[:] if h == 0 else Ys[q][:]),
                        op0=ALU.mult, op1=ALU.add)
                    if h == H - 1:
                        nc.sync.dma_start(y_d[ts(tb, 128), :], Ys[q][:])

            # ---- main interleaved schedule ----
            proj_block(0)
            dstage_prev = None
            rT_prev = None
            Ys_last = None
            for j in range(NJ):
                dstage = ypool.tile([1, H, TJ], F32, tag="dstage",
                                    name=f"dst{j}")
                if j == NJ - 1:
                    Ys_last = [ypool.tile([128, C], F32, tag=f"Yl{q}",
                                          bufs=1, name=f"Yl{q}")
                               for q in range(NQ)]
                for h in range(H):
                    if h == 5 and j + 1 < NJ:
                        proj_block(j + 1)
                    attention_head(j, h, dstage)
                    if j > 0 and h < 4:
                        if h == 0:
                            rT_prev = transp_recip(j - 1, dstage_prev)
                        out_proj_q(j - 1, h, rT_prev)
                    if j == NJ - 1 and h > 0:
                        # lag by one head: head h-1's denominators are ready
                        out_proj_head(j, h - 1, dstage, Ys_last)
                dstage_prev = dstage
            out_proj_head(NJ - 1, H - 1, dstage_prev, Ys_last)

    nc.compile()
    return nc


def _prep_inputs(x, Wq, Wk, Wv, Wp, bp):
    """Host-side shard + layout prep. Returns per-core input maps."""
    bf = ml_dtypes.bfloat16
    x = np.asarray(x, dtype=np.float32)

    def pack_w(W):  # [H, C, Dh] -> [128, NCI, H*Dh]
        Whd = np.transpose(np.asarray(W, np.float32), (1, 0, 2)).reshape(C, H * DH)
        return np.ascontiguousarray(
            Whd.reshape(NCI, 128, H * DH).transpose(1, 0, 2)
        ).astype(bf)

    wq_p, wk_p, wv_p = pack_w(Wq), pack_w(Wk), pack_w(Wv)
    wp_p = np.ascontiguousarray(
        np.asarray(Wp, np.float32).reshape(NCI, 128, C).transpose(1, 0, 2)
    ).astype(bf)

    biasb = np.broadcast_to(np.asarray(bp, np.float32), (128, C)).copy()
    iden_np = np.eye(128, dtype=np.float32)

    in_maps = []
    for b in range(B):
        xT = np.ascontiguousarray(
            x[b].T.reshape(NCI, 128, T).transpose(1, 0, 2)
        ).astype(bf)
        in_maps.append({
            "xT": xT, "wq": wq_p, "wk": wk_p, "wv": wv_p, "wp": wp_p,
            "biasb": biasb, "iden": iden_np,
        })
    return in_maps


_CACHE = {}


def kernel(x, Wq, Wk, Wv, Wp, bp):
    from concourse.bass_utils import run_bass_kernel_spmd

    if "nc" not in _CACHE:
        _CACHE["nc"] = build_kernel()
    nc = _CACHE["nc"]
    in_maps = _prep_inputs(x, Wq, Wk, Wv, Wp, bp)
    res = run_bass_kernel_spmd(nc, in_maps, list(range(NCORES)))
    out = np.stack([res.results[b]["y"] for b in range(B)], axis=0)
    return out.astype(np.float32)


# revision 21
# speedup vs baseline: 1.1340x; 1.1340x over previous
"""Multi-head causal attention (B=8, T=2048, C=384, H=6, Dh=64) on 8 TRN2 cores.

Sharding: data-parallel over batch - core b computes batch element b end to end
(no collectives).

Pipeline design (vs v1 baseline at ~300us):
- S-score psum tiles hold PAIRS of s-chunks [128, 2, 512] spanning 2 psum
  banks; ONE exp (ACT) instruction covers both chunks -> halves ACT
  per-instruction overhead (240 -> 120 exp instrs).
- exp APs are fringe-trimmed ([.., d0:512]) and PV matmuls stream only the
  causal columns (N = 512-d), removing the P memsets entirely.
- software-pipelined issue order per head: S(p) / exp(p) / PV(p-1) so the
  tensor engine always has queued work while ACT computes exp.
- FILLER QUEUE: all non-attention tensor work (QKV projection chains,
  output-projection U matmuls + normalization, denominator transposes) is
  chopped into small closures drained one-per-pair inside the attention
  inner loop.  The attention loop alone is ACT-paced (~1.04us/pair vs
  ~0.85us of PE work per pair), which idles the PE and drops its p-state
  clock from 2.4 to 1.2 GHz; the fillers keep the PE dense so everything
  runs at full clock.
- denominator row (augmented-V row 64 of O) is staged to partition 0,
  scattered to 6 partitions by one SBUF->SBUF DMA per block, then 4 batched
  transposes [6,128]->[128,6] per q-block (vs 24 single-column transposes).
  The last block uses per-head single-column transposes instead so its
  output projection pipelines per-head with no serial tail.
- output projection per (j,q): 6 per-head K=64 matmuls; normalization via
  scalar_tensor_tensor with per-partition reciprocal denominators on DVE.

Per-core layout (all "T" means transposed, head-dim/channel on partitions):
  xT   [128, 3, 2048]  bf16   c = 128*ci + p
  wq/wk[128, 3, 384]   bf16   packed Wq[h,c,d] -> [c, h*64+d]
  wv   [128, 3, 384]   bf16
  wp   [128, 3, 384]   bf16   Wp[c, e] -> [128, ci, e]
  biasb[128, 384]      f32    bias broadcast rows
  QT/KT/attT [128, 3, 2048] bf16  (hd = 128*bi + po + d, po = (h%2)*64)
  Vt   [128, 16, 6, 65] bf16  V augmented with ones col (softmax denom row)
"""

import numpy as np
import ml_dtypes

import concourse.bass as bass
import concourse.tile as tile
from concourse import bacc, mybir
from concourse.bass import ts, ds

F32 = mybir.dt.float32
BF16 = mybir.dt.bfloat16
AF = mybir.ActivationFunctionType
ALU = mybir.AluOpType

B, T, C = 8, 2048, 384
H, DH = 6, 64
SCALE = DH ** -0.5
NCORES = 8
TJ = 512            # q-block width
NJ = T // TJ        # 4 q-blocks
SC = 128            # s-chunk
NQ = TJ // SC       # q-sub-chunks / s-chunks per block (4)
NCI = C // 128      # 3 channel chunks


def build_kernel():
    nc = bacc.Bacc("TRN2", target_bir_lowering=False, debug=False)

    xT_d = nc.dram_tensor("xT", [128, NCI, T], BF16, kind="ExternalInput").ap()
    wq_d = nc.dram_tensor("wq", [128, NCI, C], BF16, kind="ExternalInput").ap()
    wk_d = nc.dram_tensor("wk", [128, NCI, C], BF16, kind="ExternalInput").ap()
    wv_d = nc.dram_tensor("wv", [128, NCI, C], BF16, kind="ExternalInput").ap()
    wp_d = nc.dram_tensor("wp", [128, NCI, C], BF16, kind="ExternalInput").ap()
    biasb_d = nc.dram_tensor("biasb", [128, C], F32, kind="ExternalInput").ap()
    iden_d = nc.dram_tensor("iden", [128, 128], F32, kind="ExternalInput").ap()
    y_d = nc.dram_tensor("y", [T, C], F32, kind="ExternalOutput").ap()

    with tile.TileContext(nc) as tc:
        with tc.tile_pool(name="const", bufs=1) as cpool, \
             tc.tile_pool(name="ps", bufs=1, space="PSUM") as ps, \
             tc.tile_pool(name="pp", bufs=3) as ppool, \
             tc.tile_pool(name="yp", bufs=2) as ypool:
            xT = cpool.tile([128, NCI, T], BF16)
            wq = cpool.tile([128, NCI, C], BF16)
            wk = cpool.tile([128, NCI, C], BF16)
            wv = cpool.tile([128, NCI, C], BF16)
            wp = cpool.tile([128, NCI, C], BF16)
            biasb = cpool.tile([128, C], F32)
            iden = cpool.tile([128, 128], F32)
            QT = cpool.tile([128, NCI, T], BF16)
            KT = cpool.tile([128, NCI, T], BF16)
            attT = cpool.tile([128, NCI, T], BF16)
            Vt = cpool.tile([128, 16, H, 65], BF16)
            onesb = cpool.tile([128, 16 * H], BF16)

            for ci in range(NCI):
                nc.sync.dma_start(xT[:, ci, :], xT_d[:, ci, :])
            nc.sync.dma_start(wq[:], wq_d[:])
            nc.sync.dma_start(wk[:], wk_d[:])
            nc.sync.dma_start(wv[:], wv_d[:])
            nc.sync.dma_start(wp[:], wp_d[:])
            nc.sync.dma_start(biasb[:], biasb_d[:])
            nc.sync.dma_start(iden[:], iden_d[:])
            # augmented-ones column of Vt (col 64 of each head slot)
            nc.gpsimd.memset(onesb[:], 1.0)
            nc.vector.tensor_copy(
                Vt[:, :, :, 64:65],
                onesb[:].rearrange("p (a b c) -> p a b c", a=16, b=H),
            )

            # deferred small PE work units, drained one per attention pair
            filler_q = []  # (label, closure)

            def drain(n=1):
                for _ in range(n):
                    if not filler_q:
                        return
                    filler_q.pop(0)[1]()

            def drain_auto():
                # drain 2 when backlogged so projections clear in time
                drain(2 if len(filler_q) > 12 else 1)

            def drain_all():
                while filler_q:
                    filler_q.pop(0)[1]()

            def drain_label(lbl):
                # force-drain (in order) until no closures tagged lbl remain
                while any(l == lbl for l, _ in filler_q):
                    drain(1)

            def enqueue_proj(jb):
                """QT/KT for t-block jb; V rows for s-chunks 4jb..4jb+3."""
                def qk_chain(dst, w, pi):
                    def run():
                        pt = ps.tile([128, TJ], F32, tag="mm", bufs=3,
                                     name=f"pqk{jb}{pi}")
                        for ci in range(NCI):
                            nc.tensor.matmul(
                                pt[:],
                                lhsT=w[:, ci, ts(pi, 128)],
                                rhs=xT[:, ci, ts(jb, TJ)],
                                start=(ci == 0), stop=(ci == NCI - 1),
                            )
                        nc.vector.tensor_copy(dst[:, pi, ts(jb, TJ)], pt[:])
                    return run

                def v_chain(si):
                    def run():
                        pt = ps.tile([128, C], F32, tag="mm", bufs=3,
                                     name=f"pv{si}")
                        for ci in range(NCI):
                            nc.tensor.matmul(
                                pt[:],
                                lhsT=xT[:, ci, ts(si, 128)],
                                rhs=wv[:, ci, :],
                                start=(ci == 0), stop=(ci == NCI - 1),
                            )
                        nc.vector.tensor_copy(
                            Vt[:, si, :, 0:64],
                            pt[:].rearrange("p (h d) -> p h d", h=H),
                        )
                    return run

                for dst, w in ((QT, wq), (KT, wk)):
                    for pi in range(NCI):
                        filler_q.append((("proj", jb), qk_chain(dst, w, pi)))
                for si in range(NQ * jb, NQ * jb + NQ):
                    filler_q.append((("proj", jb), v_chain(si)))

            def attention_head(j, h, dstage):
                po = (h % 2) * 64
                bi = h // 2
                nch = NQ * j + NQ       # s-chunks (always even)
                npair = nch // 2
                O = ps.tile([65, TJ], F32, tag="O", bufs=1, name=f"O{j}{h}")
                sps_t = [None] * npair
                P_t = [None] * npair

                def off(i):
                    return SC * i - TJ * j if i >= NQ * j else 0

                def S_pair(p):
                    spt = ps.tile([128, 2, TJ], F32, tag="sp", bufs=2,
                                  name=f"sp{j}{h}{p}")
                    sps_t[p] = spt
                    for c in (0, 1):
                        i = 2 * p + c
                        d = off(i)
                        nc.tensor.matmul(
                            spt[:, c, d:TJ],
                            lhsT=KT[po:po + 64, bi, ts(i, SC)],
                            rhs=QT[po:po + 64, bi, ds(j * TJ + d, TJ - d)],
                            start=True, stop=True,
                        )

                def EXP_pair(p):
                    d0 = off(2 * p)
                    pt = ppool.tile([128, 2, TJ], BF16, tag="P",
                                    name=f"P{j}{h}{p}")
                    P_t[p] = pt
                    nc.scalar.activation(pt[:, :, d0:TJ], sps_t[p][:, :, d0:TJ],
                                         AF.Exp, scale=SCALE)
                    for c in (0, 1):
                        i = 2 * p + c
                        if i >= NQ * j:  # diagonal window mask
                            d = off(i)
                            nc.gpsimd.affine_select(
                                out=pt[:, c, d:d + 128], in_=pt[:, c, d:d + 128],
                                pattern=[[1, 128]],
                                compare_op=ALU.is_ge,
                                fill=0.0, base=0, channel_multiplier=-1,
                            )

                def PV_pair(p):
                    for c in (0, 1):
                        i = 2 * p + c
                        d = off(i)
                        nc.tensor.matmul(
                            O[:, d:TJ],
                            lhsT=Vt[:, i, h, :],
                            rhs=P_t[p][:, c, d:TJ],
                            start=(i == 0), stop=(i == nch - 1),
                        )

                S_pair(0)
                EXP_pair(0)
                for p in range(1, npair):
                    S_pair(p)
                    EXP_pair(p)
                    drain_auto()
                    PV_pair(p - 1)
                drain_auto()
                PV_pair(npair - 1)
                # stage unnormalized attT (bf16) and the denominator row
                nc.vector.tensor_copy(attT[po:po + 64, bi, ts(j, TJ)], O[0:64, :])
                nc.vector.tensor_copy(dstage[0:1, h, :], O[64:65, :])

            def enqueue_out_proj(j, denoms):
                """Output projection for block j: batched transposes + recip,
                then per-(q, h) U matmul + normalize-accumulate closures."""
                rT_box = [None]
                Ye_box = [None] * NQ

                def transp_recip():
                    dT = ps.tile([128, NQ, H], F32, tag="mm", bufs=3,
                                 name=f"dT{j}")
                    for qq in range(NQ):
                        nc.tensor.transpose(dT[:, qq, :],
                                            denoms[0:H, ts(qq, 128)],
                                            iden[0:H, 0:H])
                    rT = ypool.tile([128, NQ, H], F32, tag="rT", name=f"rT{j}")
                    nc.vector.reciprocal(rT[:], dT[:])
                    rT_box[0] = rT
                filler_q.append((("outp", j), transp_recip))

                def u_step(q, h):
                    def run():
                        tb = NQ * j + q
                        po = (h % 2) * 64
                        bi = h // 2
                        if h == 0:
                            Ye_box[q] = ypool.tile([128, C], F32, tag="Ye",
                                                   name=f"Ye{tb}")
                        Ye = Ye_box[q]
                        U = ps.tile([128, C], F32, tag="mm", bufs=3,
                                    name=f"U{tb}{h}")
                        nc.tensor.matmul(
                            U[:],
                            lhsT=attT[po:po + 64, bi, ts(tb, 128)],
                            rhs=wp[po:po + 64, bi, :],
                            start=True, stop=True,
                        )
                        nc.vector.scalar_tensor_tensor(
                            out=Ye[:], in0=U[:], scalar=rT_box[0][:, q, h:h + 1],
                            in1=(biasb[:] if h == 0 else Ye[:]),
                            op0=ALU.mult, op1=ALU.add)
                        if h == H - 1:
                            nc.sync.dma_start(y_d[ts(tb, 128), :], Ye[:])
                    return run

                for q in range(NQ):
                    for h in range(H):
                        filler_q.append((("outp", j), u_step(q, h)))

            def enqueue_out_proj_last(j, h, dstage, Ys, rh_box):
                """Per-head output projection pieces for the LAST block."""
                def transp_recip_h():
                    dTh = ps.tile([128, NQ], F32, tag="mm", bufs=3,
                                  name=f"dTh{j}{h}")
                    for qq in range(NQ):
                        nc.tensor.transpose(dTh[:, qq:qq + 1],
                                            dstage[0:1, h, ts(qq, 128)],
                                            iden[0:1, 0:1])
                    rh = ypool.tile([128, NQ], F32, tag="rh", name=f"rh{j}{h}")
                    nc.vector.reciprocal(rh[:], dTh[:])
                    rh_box[0] = rh
                filler_q.append((("outl", h), transp_recip_h))

                def u_step(q):
                    def run():
                        tb = NQ * j + q
                        po = (h % 2) * 64
                        bi = h // 2
                        U = ps.tile([128, C], F32, tag="mm", bufs=3,
                                    name=f"Ul{tb}{h}")
                        nc.tensor.matmul(
                            U[:],
                            lhsT=attT[po:po + 64, bi, ts(tb, 128)],
                            rhs=wp[po:po + 64, bi, :],
                            start=True, stop=True,
                        )
                        nc.vector.scalar_tensor_tensor(
                            out=Ys[q][:], in0=U[:], scalar=rh_box[0][:, q:q + 1],
                            in1=(biasb[:] if h == 0 else Ys[q][:]),
                            op0=ALU.mult, op1=ALU.add)
                        if h == H - 1:
                            nc.sync.dma_start(y_d[ts(tb, 128), :], Ys[q][:])
                    return run

                for q in range(NQ):
                    filler_q.append((("outl", h), u_step(q)))

            # ---- main schedule ----
            # prologue: block-0 projections run directly (nothing to overlap)
            enqueue_proj(0)
            drain_all()
            dstage_prev = None
            Ys_last = None
            for j in range(NJ):
                dstage = ypool.tile([1, H, TJ], F32, tag="dstage",
                                    name=f"dst{j}")
                if j == NJ - 1:
                    Ys_last = [ypool.tile([128, C], F32, tag=f"Yl{q}",
                                          bufs=1, name=f"Yl{q}")
                               for q in range(NQ)]
                    rh_boxes = [[None] for _ in range(H)]
                # any projection fillers for THIS block must have issued
                # before its S matmuls read QT/KT
                drain_label(("proj", j))
                for h in range(H):
                    if j > 0 and h == 0:
                        # previous block's denominators: scatter to 6
                        # partitions and queue its output projection
                        denoms = ypool.tile([H, TJ], F32, tag="denoms",
                                            name=f"den{j - 1}")
                        nc.sync.dma_start(denoms[0:H, :],
                                          dstage_prev[0:1, :, :])
                        if j < NJ - 1 or True:
                            pass
                        enqueue_out_proj(j - 1, denoms)
                    attention_head(j, h, dstage)
                    if h == 1 and j + 1 < NJ:
                        enqueue_proj(j + 1)
                    if j == NJ - 1:
                        enqueue_out_proj_last(j, h, dstage, Ys_last,
                                              rh_boxes[h])
                dstage_prev = dstage
            drain_all()

    nc.compile()
    return nc


def _prep_inputs(x, Wq, Wk, Wv, Wp, bp):
    """Host-side shard + layout prep. Returns per-core input maps."""
    bf = ml_dtypes.bfloat16
    x = np.asarray(x, dtype=np.float32)

    def pack_w(W):  # [H, C, Dh] -> [128, NCI, H*Dh]
        Whd = np.transpose(np.asarray(W, np.float32), (1, 0, 2)).reshape(C, H * DH)
        return np.ascontiguousarray(
            Whd.reshape(NCI, 128, H * DH).transpose(1, 0, 2)
        ).astype(bf)

    wq_p, wk_p, wv_p = pack_w(Wq), pack_w(Wk), pack_w(Wv)
    wp_p = np.ascontiguousarray(
        np.asarray(Wp, np.float32).reshape(NCI, 128, C).transpose(1, 0, 2)
    ).astype(bf)

    biasb = np.broadcast_to(np.asarray(bp, np.float32), (128, C)).copy()
    iden_np = np.eye(128, dtype=np.float32)

    in_maps = []
    for b in range(B):
        xT = np.ascontiguousarray(
            x[b].T.reshape(NCI, 128, T).transpose(1, 0, 2)
        ).astype(bf)
        in_maps.append({
            "xT": xT, "wq": wq_p, "wk": wk_p, "wv": wv_p, "wp": wp_p,
            "biasb": biasb, "iden": iden_np,
        })
    return in_maps


_CACHE = {}


def kernel(x, Wq, Wk, Wv, Wp, bp):
    from concourse.bass_utils import run_bass_kernel_spmd

    if "nc" not in _CACHE:
        _CACHE["nc"] = build_kernel()
    nc = _CACHE["nc"]
    in_maps = _prep_inputs(x, Wq, Wk, Wv, Wp, bp)
    res = run_bass_kernel_spmd(nc, in_maps, list(range(NCORES)))
    out = np.stack([res.results[b]["y"] for b in range(B)], axis=0)
    return out.astype(np.float32)


# revision 22
# speedup vs baseline: 1.1529x; 1.0166x over previous
"""Multi-head causal attention (B=8, T=2048, C=384, H=6, Dh=64) on 8 TRN2 cores.

Sharding: data-parallel over batch - core b computes batch element b end to end
(no collectives).

Pipeline design (vs v1 baseline at ~300us):
- S-score psum tiles hold PAIRS of s-chunks [128, 2, 512] spanning 2 psum
  banks; ONE exp (ACT) instruction covers both chunks -> halves ACT
  per-instruction overhead (240 -> 120 exp instrs).
- exp APs are fringe-trimmed ([.., d0:512]) and PV matmuls stream only the
  causal columns (N = 512-d), removing the P memsets entirely.
- software-pipelined issue order per head: S(p) / exp(p) / PV(p-1) so the
  tensor engine always has queued work while ACT computes exp.
- FILLER QUEUE: all non-attention tensor work (QKV projection chains,
  output-projection U matmuls + normalization, denominator transposes) is
  chopped into small closures drained one-per-pair inside the attention
  inner loop.  The attention loop alone is ACT-paced (~1.04us/pair vs
  ~0.85us of PE work per pair), which idles the PE and drops its p-state
  clock from 2.4 to 1.2 GHz; the fillers keep the PE dense so everything
  runs at full clock.
- denominator row (augmented-V row 64 of O) is staged to partition 0,
  scattered to 6 partitions by one SBUF->SBUF DMA per block, then 4 batched
  transposes [6,128]->[128,6] per q-block (vs 24 single-column transposes).
  The last block uses per-head single-column transposes instead so its
  output projection pipelines per-head with no serial tail.
- output projection per (j,q): 6 per-head K=64 matmuls; normalization via
  scalar_tensor_tensor with per-partition reciprocal denominators on DVE.

Per-core layout (all "T" means transposed, head-dim/channel on partitions):
  xT   [128, 3, 2048]  bf16   c = 128*ci + p
  wq/wk[128, 3, 384]   bf16   packed Wq[h,c,d] -> [c, h*64+d]
  wv   [128, 3, 384]   bf16
  wp   [128, 3, 384]   bf16   Wp[c, e] -> [128, ci, e]
  biasb[128, 384]      f32    bias broadcast rows
  QT/KT/attT [128, 3, 2048] bf16  (hd = 128*bi + po + d, po = (h%2)*64)
  Vt   [128, 16, 6, 65] bf16  V augmented with ones col (softmax denom row)
"""

import numpy as np
import ml_dtypes

import concourse.bass as bass
import concourse.tile as tile
from concourse import bacc, mybir
from concourse.bass import ts, ds

F32 = mybir.dt.float32
BF16 = mybir.dt.bfloat16
AF = mybir.ActivationFunctionType
ALU = mybir.AluOpType

B, T, C = 8, 2048, 384
H, DH = 6, 64
SCALE = DH ** -0.5
NCORES = 8
TJ = 512            # q-block width
NJ = T // TJ        # 4 q-blocks
SC = 128            # s-chunk
NQ = TJ // SC       # q-sub-chunks / s-chunks per block (4)
NCI = C // 128      # 3 channel chunks


def build_kernel():
    nc = bacc.Bacc("TRN2", target_bir_lowering=False, debug=False)

    xT_d = nc.dram_tensor("xT", [128, NCI, T], BF16, kind="ExternalInput").ap()
    wq_d = nc.dram_tensor("wq", [128, NCI, C], BF16, kind="ExternalInput").ap()
    wk_d = nc.dram_tensor("wk", [128, NCI, C], BF16, kind="ExternalInput").ap()
    wv_d = nc.dram_tensor("wv", [128, NCI, C], BF16, kind="ExternalInput").ap()
    wp_d = nc.dram_tensor("wp", [128, NCI, C], BF16, kind="ExternalInput").ap()
    biasb_d = nc.dram_tensor("biasb", [128, C], F32, kind="ExternalInput").ap()
    iden_d = nc.dram_tensor("iden", [128, 128], F32, kind="ExternalInput").ap()
    y_d = nc.dram_tensor("y", [T, C], F32, kind="ExternalOutput").ap()

    with tile.TileContext(nc) as tc:
        with tc.tile_pool(name="const", bufs=1) as cpool, \
             tc.tile_pool(name="ps", bufs=1, space="PSUM") as ps, \
             tc.tile_pool(name="pp", bufs=3) as ppool, \
             tc.tile_pool(name="yp", bufs=2) as ypool:
            xT = cpool.tile([128, NCI, T], BF16)
            wq = cpool.tile([128, NCI, C], BF16)
            wk = cpool.tile([128, NCI, C], BF16)
            wv = cpool.tile([128, NCI, C], BF16)
            wp = cpool.tile([128, NCI, C], BF16)
            biasb = cpool.tile([128, C], F32)
            iden = cpool.tile([128, 128], F32)
            QT = cpool.tile([128, NCI, T], BF16)
            KT = cpool.tile([128, NCI, T], BF16)
            attT = cpool.tile([128, NCI, T], BF16)
            Vt = cpool.tile([128, 16, H, 65], BF16)
            onesb = cpool.tile([128, 16 * H], BF16)

            for ci in range(NCI):
                nc.sync.dma_start(xT[:, ci, :], xT_d[:, ci, :])
            nc.sync.dma_start(wq[:], wq_d[:])
            nc.sync.dma_start(wk[:], wk_d[:])
            nc.sync.dma_start(wv[:], wv_d[:])
            nc.sync.dma_start(wp[:], wp_d[:])
            nc.sync.dma_start(biasb[:], biasb_d[:])
            nc.sync.dma_start(iden[:], iden_d[:])
            # augmented-ones column of Vt (col 64 of each head slot)
            nc.gpsimd.memset(onesb[:], 1.0)
            nc.vector.tensor_copy(
                Vt[:, :, :, 64:65],
                onesb[:].rearrange("p (a b c) -> p a b c", a=16, b=H),
            )

            # deferred small PE work units, drained one per attention pair
            filler_q = []  # (label, closure)

            def drain(n=1):
                for _ in range(n):
                    if not filler_q:
                        return
                    filler_q.pop(0)[1]()

            def drain_auto():
                # drain 2 when backlogged so projections clear in time
                drain(2 if len(filler_q) > 12 else 1)

            def drain_all():
                while filler_q:
                    filler_q.pop(0)[1]()

            def drain_label(lbl):
                # force-drain (in order) until no closures tagged lbl remain
                while any(l == lbl for l, _ in filler_q):
                    drain(1)

            def enqueue_proj(jb):
                """QT/KT for t-block jb; V rows for s-chunks 4jb..4jb+3."""
                def qk_chain(dst, w, pi):
                    def run():
                        pt = ps.tile([128, TJ], F32, tag="mm", bufs=2,
                                     name=f"pqk{jb}{pi}")
                        for ci in range(NCI):
                            nc.tensor.matmul(
                                pt[:],
                                lhsT=w[:, ci, ts(pi, 128)],
                                rhs=xT[:, ci, ts(jb, TJ)],
                                start=(ci == 0), stop=(ci == NCI - 1),
                            )
                        nc.vector.tensor_copy(dst[:, pi, ts(jb, TJ)], pt[:])
                    return run

                def v_chain(si):
                    def run():
                        pt = ps.tile([128, C], F32, tag="mm", bufs=2,
                                     name=f"pv{si}")
                        for ci in range(NCI):
                            nc.tensor.matmul(
                                pt[:],
                                lhsT=xT[:, ci, ts(si, 128)],
                                rhs=wv[:, ci, :],
                                start=(ci == 0), stop=(ci == NCI - 1),
                            )
                        nc.vector.tensor_copy(
                            Vt[:, si, :, 0:64],
                            pt[:].rearrange("p (h d) -> p h d", h=H),
                        )
                    return run

                filler_q.append((("projp", jb, 0), qk_chain(QT, wq, 0)))
                filler_q.append((("projp", jb, 0), qk_chain(KT, wk, 0)))
                for si in range(NQ * jb, NQ * jb + NQ):
                    filler_q.append((("projv", jb), v_chain(si)))
                for pi in (1, 2):
                    filler_q.append((("projp", jb, pi), qk_chain(QT, wq, pi)))
                    filler_q.append((("projp", jb, pi), qk_chain(KT, wk, pi)))

            def attention_head(j, h, dstage):
                po = (h % 2) * 64
                bi = h // 2
                nch = NQ * j + NQ       # s-chunks (always even)
                npair = nch // 2
                O = ps.tile([65, TJ], F32, tag="O", bufs=2, name=f"O{j}{h}")
                sps_t = [None] * npair
                P_t = [None] * npair

                def off(i):
                    return SC * i - TJ * j if i >= NQ * j else 0

                def S_pair(p):
                    spt = ps.tile([128, 2, TJ], F32, tag="sp", bufs=2,
                                  name=f"sp{j}{h}{p}")
                    sps_t[p] = spt
                    for c in (0, 1):
                        i = 2 * p + c
                        d = off(i)
                        nc.tensor.matmul(
                            spt[:, c, d:TJ],
                            lhsT=KT[po:po + 64, bi, ts(i, SC)],
                            rhs=QT[po:po + 64, bi, ds(j * TJ + d, TJ - d)],
                            start=True, stop=True,
                        )

                def EXP_pair(p):
                    d0 = off(2 * p)
                    pt = ppool.tile([128, 2, TJ], BF16, tag="P",
                                    name=f"P{j}{h}{p}")
                    P_t[p] = pt
                    nc.scalar.activation(pt[:, :, d0:TJ], sps_t[p][:, :, d0:TJ],
                                         AF.Exp, scale=SCALE)
                    for c in (0, 1):
                        i = 2 * p + c
                        if i >= NQ * j:  # diagonal window mask
                            d = off(i)
                            nc.gpsimd.affine_select(
                                out=pt[:, c, d:d + 128], in_=pt[:, c, d:d + 128],
                                pattern=[[1, 128]],
                                compare_op=ALU.is_ge,
                                fill=0.0, base=0, channel_multiplier=-1,
                            )

                def PV_pair(p):
                    for c in (0, 1):
                        i = 2 * p + c
                        d = off(i)
                        nc.tensor.matmul(
                            O[:, d:TJ],
                            lhsT=Vt[:, i, h, :],
                            rhs=P_t[p][:, c, d:TJ],
                            start=(i == 0), stop=(i == nch - 1),
                        )

                S_pair(0)
                EXP_pair(0)
                for p in range(1, npair):
                    S_pair(p)
                    EXP_pair(p)
                    drain_auto()
                    PV_pair(p - 1)
                drain_auto()
                PV_pair(npair - 1)
                # stage unnormalized attT (bf16) and the denominator row
                nc.vector.tensor_copy(attT[po:po + 64, bi, ts(j, TJ)], O[0:64, :])
                nc.vector.tensor_copy(dstage[0:1, h, :], O[64:65, :])

            def enqueue_out_proj(j, denoms):
                """Output projection for block j: batched transposes + recip,
                then per-(q, h) U matmul + normalize-accumulate closures."""
                rT_box = [None]
                Ye_box = [None] * NQ

                def transp_recip():
                    dT = ps.tile([128, NQ, H], F32, tag="mm", bufs=2,
                                 name=f"dT{j}")
                    for qq in range(NQ):
                        nc.tensor.transpose(dT[:, qq, :],
                                            denoms[0:H, ts(qq, 128)],
                                            iden[0:H, 0:H])
                    rT = ypool.tile([128, NQ, H], F32, tag="rT", name=f"rT{j}")
                    nc.vector.reciprocal(rT[:], dT[:])
                    rT_box[0] = rT
                filler_q.append((("outp", j), transp_recip))

                def u_step(q, h):
                    def run():
                        tb = NQ * j + q
                        po = (h % 2) * 64
                        bi = h // 2
                        if h == 0:
                            Ye_box[q] = ypool.tile([128, C], F32, tag="Ye",
                                                   name=f"Ye{tb}")
                        Ye = Ye_box[q]
                        U = ps.tile([128, C], F32, tag="mm", bufs=2,
                                    name=f"U{tb}{h}")
                        nc.tensor.matmul(
                            U[:],
                            lhsT=attT[po:po + 64, bi, ts(tb, 128)],
                            rhs=wp[po:po + 64, bi, :],
                            start=True, stop=True,
                        )
                        nc.vector.scalar_tensor_tensor(
                            out=Ye[:], in0=U[:], scalar=rT_box[0][:, q, h:h + 1],
                            in1=(biasb[:] if h == 0 else Ye[:]),
                            op0=ALU.mult, op1=ALU.add)
                        if h == H - 1:
                            nc.sync.dma_start(y_d[ts(tb, 128), :], Ye[:])
                    return run

                for q in range(NQ):
                    for h in range(H):
                        filler_q.append((("outp", j), u_step(q, h)))

            def enqueue_out_proj_last(j, h, dstage, Ys, rh_box):
                """Per-head output projection pieces for the LAST block."""
                def transp_recip_h():
                    dTh = ps.tile([128, NQ], F32, tag="mm", bufs=2,
                                  name=f"dTh{j}{h}")
                    for qq in range(NQ):
                        nc.tensor.transpose(dTh[:, qq:qq + 1],
                                            dstage[0:1, h, ts(qq, 128)],
                                            iden[0:1, 0:1])
                    rh = ypool.tile([128, NQ], F32, tag="rh", name=f"rh{j}{h}")
                    nc.vector.reciprocal(rh[:], dTh[:])
                    rh_box[0] = rh
                filler_q.append((("outl", h), transp_recip_h))

                def u_step(q):
                    def run():
                        tb = NQ * j + q
                        po = (h % 2) * 64
                        bi = h // 2
                        U = ps.tile([128, C], F32, tag="mm", bufs=2,
                                    name=f"Ul{tb}{h}")
                        nc.tensor.matmul(
                            U[:],
                            lhsT=attT[po:po + 64, bi, ts(tb, 128)],
                            rhs=wp[po:po + 64, bi, :],
                            start=True, stop=True,
                        )
                        nc.vector.scalar_tensor_tensor(
                            out=Ys[q][:], in0=U[:], scalar=rh_box[0][:, q:q + 1],
                            in1=(biasb[:] if h == 0 else Ys[q][:]),
                            op0=ALU.mult, op1=ALU.add)
                        if h == H - 1:
                            nc.sync.dma_start(y_d[ts(tb, 128), :], Ys[q][:])
                    return run

                for q in range(NQ):
                    filler_q.append((("outl", h), u_step(q)))

            # ---- main schedule ----
            enqueue_proj(0)
            dstage_prev = None
            Ys_last = None
            for j in range(NJ):
                dstage = ypool.tile([1, H, TJ], F32, tag="dstage",
                                    name=f"dst{j}")
                if j == NJ - 1:
                    Ys_last = [ypool.tile([128, C], F32, tag=f"Yl{q}",
                                          bufs=1, name=f"Yl{q}")
                               for q in range(NQ)]
                    rh_boxes = [[None] for _ in range(H)]
                for h in range(H):
                    # guards: this head's QT/KT channel chunk and (for the
                    # first head) the block's V rows must be issued already
                    if h == 0:
                        drain_label(("projv", j))
                    drain_label(("projp", j, h // 2))
                    if j > 0 and h == 0:
                        enqueue_out_proj(j - 1, denoms_prev)
                    attention_head(j, h, dstage)
                    if h == 1 and j + 1 < NJ:
                        enqueue_proj(j + 1)
                    if j == NJ - 1:
                        enqueue_out_proj_last(j, h, dstage, Ys_last,
                                              rh_boxes[h])
                # scatter this block's denominators now: the SBUF->SBUF DMA
                # (~2us latency) completes while the next block's first head
                # runs, so the transpose filler never waits on it
                denoms_prev = ypool.tile([H, TJ], F32, tag="denoms",
                                         name=f"den{j}")
                nc.sync.dma_start(denoms_prev[0:H, :], dstage[0:1, :, :])
                dstage_prev = dstage
            drain_all()

    nc.compile()
    return nc


def _prep_inputs(x, Wq, Wk, Wv, Wp, bp):
    """Host-side shard + layout prep. Returns per-core input maps."""
    bf = ml_dtypes.bfloat16
    x = np.asarray(x, dtype=np.float32)

    def pack_w(W):  # [H, C, Dh] -> [128, NCI, H*Dh]
        Whd = np.transpose(np.asarray(W, np.float32), (1, 0, 2)).reshape(C, H * DH)
        return np.ascontiguousarray(
            Whd.reshape(NCI, 128, H * DH).transpose(1, 0, 2)
        ).astype(bf)

    wq_p, wk_p, wv_p = pack_w(Wq), pack_w(Wk), pack_w(Wv)
    wp_p = np.ascontiguousarray(
        np.asarray(Wp, np.float32).reshape(NCI, 128, C).transpose(1, 0, 2)
    ).astype(bf)

    biasb = np.broadcast_to(np.asarray(bp, np.float32), (128, C)).copy()
    iden_np = np.eye(128, dtype=np.float32)

    in_maps = []
    for b in range(B):
        xT = np.ascontiguousarray(
            x[b].T.reshape(NCI, 128, T).transpose(1, 0, 2)
        ).astype(bf)
        in_maps.append({
            "xT": xT, "wq": wq_p, "wk": wk_p, "wv": wv_p, "wp": wp_p,
            "biasb": biasb, "iden": iden_np,
        })
    return in_maps


_CACHE = {}


def kernel(x, Wq, Wk, Wv, Wp, bp):
    from concourse.bass_utils import run_bass_kernel_spmd

    if "nc" not in _CACHE:
        _CACHE["nc"] = build_kernel()
    nc = _CACHE["nc"]
    in_maps = _prep_inputs(x, Wq, Wk, Wv, Wp, bp)
    res = run_bass_kernel_spmd(nc, in_maps, list(range(NCORES)))
    out = np.stack([res.results[b]["y"] for b in range(B)], axis=0)
    return out.astype(np.float32)
